# revision 23
# baseline (speedup 1.0000x reference)
"""Trainium2 Bass kernel for nn_ConvShiftLayer.

Computes, per batch element n:
    h = x[n] @ W_dense + b_dense                      (2048, 2048)
    y[t, o] = sum_{d=0..7} h[t-d, (o+d) % 2048]       (h[<0] = 0)
    a = tanh(y),  z = (y > 0) as f32
Returns (y, a, z) each of shape (8, 2048, 2048) f32.

Strategy: data-parallel over batch, 1 element per NeuronCore (8 cores).
Per core:
  - x is PE-transposed to xT tiles (D on partitions), 4 transposes per
    PSUM bank, copies split across scalar/vector engines.
  - h = xT.T @ W via fp32r matmuls (full-rate) into PSUM, copied to SBUF
    by the Scalar engine (activation Copy, f32r out); tiles hold 128
    overlapping time rows (stride 121) plus 1 wrap column.
  - The 8-tap shifted sum is a 3-stage shift tree:
        s1 = h  + Z h      (Z = time -1, chan +1)
        s2 = s1 + Z^2 s1
        y  = s2 + Z^4 s2 (+ bias)
    Each stage: 1 PE matmul for the shifted term (shift matrix
    stationary, channel shift via rhs free offset) + 1 DVE add for the
    identity term.  Cuts tap PE cost from 9 to 3-4 matmuls per chunk.
  - a = tanh on ScalarE, z = is_gt on DVE (2x SBUF mode).
  - Outputs staged in [128, 1024] pair-tiles -> 4 KB DMA lines, issued
    from sync (y), scalar (a) and gpsimd (z) queues; last tile fans its
    chunk DMAs across all four queues to shorten the tail drain.
"""

import sys

if "/opt/trn_rl_repo" not in sys.path:
    sys.path.insert(0, "/opt/trn_rl_repo")

import numpy as np

B, L, DIN, F = 8, 2048, 1024, 2048
WC = 8            # conv taps
PAD = WC - 1      # 7
TS = 128 - PAD    # 121 output rows per time tile
NT = (L + TS - 1) // TS   # 17 time tiles
NCH = 4           # channel chunks of 512
CW = 512          # chunk width
NCORES = 8
HW_ = F + 4       # h/s1/s2 tile width (max wrap need is s2: +4)

# consts tensor column layout (one [128, 2816] f32 input)
#   [0:128)      T1 down-shift by 1  (T1[m-1, m] = 1)
#   [128:256)    T2 down-shift by 2
#   [256:384)    T4 down-shift by 4
#   [384:512)    ones8 block  (rows 0..7 = 1)       -- bias, tiles i>0
#   [512:640)    L0 triangular (row d: 1 for cols >= d) -- bias, tile 0
#   [640:768)    identity (PE transpose)
#   [768:2816)   bsh: row d = bias shifted by d (rows 8..127 zero)
CONST_COLS = 768 + F

_CACHE = {}


def _build_consts(b):
    c = np.zeros((128, CONST_COLS), np.float32)
    for sh, base in ((1, 0), (2, 128), (4, 256)):
        for m in range(sh, 128):
            c[m - sh, base + m] = 1.0
    c[0:WC, 384:512] = 1.0
    for d in range(WC):
        c[d, 512 + d:640] = 1.0
    c[:, 640:768] = np.eye(128, dtype=np.float32)
    bext = np.concatenate([b, b[: WC - 1]])
    for d in range(WC):
        c[d, 768:768 + F] = bext[d:d + F]
    return c


def _split_matmul_waits(nc):
    """This walrus build accepts only one sync-wait command per instruction;
    hoist extra waits onto preceding same-engine no-ops (one wait each)."""
    import concourse.mybir as mybir

    for fn in nc.m.functions:
        for blk in fn.blocks:
            newl = []
            for inst in blk.instructions:
                si = getattr(inst, "sync_info", None)
                if (
                    si is not None
                    and len(si.on_wait) > 1
                    and not isinstance(inst, mybir.InstNoOp)
                    and getattr(inst, "engine", None) is not None
                ):
                    waits = list(si.on_wait)
                    for wi, w in enumerate(waits[:-1]):
                        pre = mybir.InstNoOp(
                            name=f"{inst.name}_wsplit{wi}",
                            sync_info=mybir.SyncInfo(on_wait=[w], on_update=[]),
                            bass_nofuse=True,
                            engine=inst.engine,
                        )
                        newl.append(pre)
                    si.on_wait = waits[-1:]
                newl.append(inst)
            blk.instructions = newl


def _build_nc(use_bias, mm_dtype_name="float32r", split_waits=True):
    import concourse.bass as bass
    import concourse.mybir as mybir
    from concourse import tile

    f32 = mybir.dt.float32
    mmdt = getattr(mybir.dt, mm_dtype_name)
    Act = mybir.ActivationFunctionType

    nc = bass.Bass("TRN2", target_bir_lowering=False, debug=False)

    x_d = nc.declare_dram_parameter("x", [L, DIN], f32, isOutput=False)
    w_d = nc.declare_dram_parameter("w", [DIN, F], f32, isOutput=False)
    cst_d = nc.declare_dram_parameter("cst", [128, CONST_COLS], f32, isOutput=False)
    y_d = nc.declare_dram_parameter("y", [L, F], f32, isOutput=True)
    a_d = nc.declare_dram_parameter("a", [L, F], f32, isOutput=True)
    z_d = nc.declare_dram_parameter("z", [L, F], f32, isOutput=True)

    KD = DIN // 128  # 8 K-tiles
    CCOLS = CONST_COLS if use_bias else 768

    with tile.TileContext(nc) as tc:
        with (
            tc.tile_pool(name="wpool", bufs=1) as wpool,
            tc.tile_pool(name="xtpool", bufs=1) as xtpool,
            tc.tile_pool(name="cpool", bufs=1) as cpool,
        ):
            cst = cpool.tile([128, CCOLS], mmdt, tag="cst", name="cst")
            wt = [wpool.tile([128, F], mmdt, tag=f"w{k}", name=f"w{k}") for k in range(KD)]
            xt = [
                xtpool.tile([128, L], mmdt, tag=f"xt{k}", name=f"xt{k}")
                for k in range(KD)
            ]

            ident = cst[:, 640:768].bitcast(f32)

            # ---- phase 0: transpose x (L,DIN) -> xT tiles [128 D, L t] ----
            with (
                tc.tile_pool(name="xstage", bufs=3) as xstage,
                tc.tile_pool(name="psum0", bufs=2, space="PSUM") as psum0,
            ):
                for i in range(L // 128):
                    xs = xstage.tile([128, DIN], f32, tag="xs")
                    if i == 0:
                        qe = [nc.sync, nc.gpsimd, nc.sync, nc.gpsimd]
                        for q in range(4):
                            qe[q].dma_start(
                                xs[:, q * 256:(q + 1) * 256],
                                x_d[0:128, q * 256:(q + 1) * 256],
                            )
                    else:
                        nc.sync.dma_start(xs[:], x_d[i * 128:(i + 1) * 128, :])
                    if i == 0:
                        # W/cst loads issued after x0 so transposes start
                        # immediately; they ride parallel queues.
                        nc.gpsimd.dma_start(
                            cst[:], cst_d[:, 0:CCOLS].bitcast(mmdt)
                        )
                        for k in range(KD):
                            nc.scalar.dma_start(
                                wt[k][:],
                                w_d[k * 128:(k + 1) * 128, :].bitcast(mmdt),
                            )
                    for g in range(2):  # groups of 4 transposes per bank
                        tp = psum0.tile([128, 512], f32, tag="tp")
                        for j in range(4):
                            k = g * 4 + j
                            nc.tensor.transpose(
                                tp[:, j * 128:(j + 1) * 128],
                                xs[:, k * 128:(k + 1) * 128],
                                ident,
                            )
                        for j in range(4):
                            k = g * 4 + j
                            dst = xt[k][:, i * 128:(i + 1) * 128]
                            src = tp[:, j * 128:(j + 1) * 128]
                            if k % 2 == 0 and i > 0:
                                nc.scalar.activation(dst, src, Act.Copy)
                            else:
                                # i == 0: all copies on the (idle) DVE so the
                                # W DGE setups on scalar don't block the
                                # first transpose batch.
                                nc.vector.tensor_copy(dst, src)

            # ---- phase 1: software-pipelined over time tiles ----
            # Emission order: dense matmuls of tile ii, then tap stages of
            # tile ii-1.  Keeps ~8us of independent dense work queued ahead
            # of every cross-engine-dependent stage matmul so the in-order
            # PE queue never stalls on DVE/scalar sems (head-of-line block).
            with (
                tc.tile_pool(name="hring", bufs=4) as hring,
                tc.tile_pool(name="opool", bufs=3) as opool,
                tc.tile_pool(name="psum_h", bufs=3, space="PSUM") as psum_h,
                tc.tile_pool(name="psum_m", bufs=5, space="PSUM") as psum_m,
            ):
                def tile_geom(i):
                    t0 = TS * i
                    My = min(TS, L - t0)
                    if i == 0:
                        return t0, My, 0, 128, 0
                    hlo = t0 - PAD
                    Mh = min(L, t0 + TS) - hlo
                    return t0, My, hlo, Mh, PAD

                for i in range(NT):
                    t0, My, hlo, Mh, lo = tile_geom(i)
                    hs = hring.tile([128, HW_], f32, tag="h", name=f"hs{i}")
                    s1 = hring.tile([128, HW_], f32, tag="h", name=f"s1_{i}")
                    s2 = hring.tile([128, HW_], f32, tag="h", name=f"s2_{i}")
                    hsr = hs.bitcast(mmdt)
                    s1r = s1.bitcast(mmdt)
                    s2r = s2.bitcast(mmdt)

                    # h = xT.T @ W ; PSUM -> SBUF on scalar engine
                    for n in range(NCH):
                        hp = psum_h.tile([128, CW], f32, tag="hp")
                        for k in range(KD):
                            nc.tensor.matmul(
                                hp[0:Mh, :],
                                xt[k][:, hlo:hlo + Mh],
                                wt[k][:, n * CW:(n + 1) * CW],
                                start=(k == 0),
                                stop=(k == KD - 1),
                            )
                        nc.scalar.activation(
                            hsr[0:Mh, n * CW:(n + 1) * CW], hp[0:Mh, :], Act.Copy
                        )
                        if n == 0:  # wrap col: hs[:, F] = hs[:, 0]
                            nc.scalar.activation(
                                hsr[0:Mh, F:F + 1], hsr[0:Mh, 0:1], Act.Copy
                            )

                    # stage 1: s1 = hs + Z hs
                    for n in range(NCH):
                        c0 = n * CW
                        m1 = psum_m.tile([128, CW], f32, tag="m")
                        nc.tensor.matmul(
                            m1[0:Mh, :],
                            cst[0:Mh, 0:Mh],
                            hsr[0:Mh, c0 + 1:c0 + 1 + CW],
                            start=True,
                            stop=True,
                        )
                        nc.vector.tensor_tensor(
                            s1r[0:Mh, c0:c0 + CW],
                            hs[0:Mh, c0:c0 + CW],
                            m1[0:Mh, :],
                            mybir.AluOpType.add,
                        )
                        if n == 0:  # wrap cols: s1[:, F:F+2] = s1[:, 0:2]
                            nc.scalar.activation(
                                s1r[0:Mh, F:F + 2], s1r[0:Mh, 0:2], Act.Copy
                            )

                    # stage 2: s2 = s1 + Z^2 s1
                    for n in range(NCH):
                        c0 = n * CW
                        m2 = psum_m.tile([128, CW], f32, tag="m")
                        nc.tensor.matmul(
                            m2[0:Mh, :],
                            cst[0:Mh, 128:128 + Mh],
                            s1r[0:Mh, c0 + 2:c0 + 2 + CW],
                            start=True,
                            stop=True,
                        )
                        nc.vector.tensor_tensor(
                            s2r[0:Mh, c0:c0 + CW],
                            s1[0:Mh, c0:c0 + CW],
                            m2[0:Mh, :],
                            mybir.AluOpType.add,
                        )
                        if n == 0:  # wrap cols: s2[:, F:F+4] = s2[:, 0:4]
                            nc.scalar.activation(
                                s2r[0:Mh, F:F + 4], s2r[0:Mh, 0:4], Act.Copy
                            )

                    # stage 3: y = s2 + Z^4 s2 (+ bias); tanh; is_gt; DMA out
                    bias_col = 512 if i == 0 else 384
                    last = i >= NT - 3
                    ys = as_ = zs = None
                    for n in range(NCH):
                        c0 = n * CW
                        if n % 2 == 0:
                            ys = opool.tile([128, 2 * CW], f32, tag="ys", name="ys")
                            as_ = opool.tile([128, 2 * CW], f32, tag="as", name="as_")
                            zs = opool.tile([128, 2 * CW], f32, tag="zs", name="zs")
                        h0 = (n % 2) * CW
                        m3 = psum_m.tile([128, CW], f32, tag="m")
                        nc.tensor.matmul(
                            m3[0:Mh, :],
                            cst[0:Mh, 256:256 + Mh],
                            s2r[0:Mh, c0 + 4:c0 + 4 + CW],
                            start=True,
                            stop=not use_bias,
                        )
                        if use_bias:
                            nc.tensor.matmul(
                                m3[0:Mh, :],
                                cst[0:Mh, bias_col:bias_col + Mh],
                                cst[0:Mh, 768 + c0:768 + c0 + CW],
                                start=False,
                                stop=True,
                            )
                        nc.vector.tensor_tensor(
                            ys[0:Mh, h0:h0 + CW],
                            s2[0:Mh, c0:c0 + CW],
                            m3[0:Mh, :],
                            mybir.AluOpType.add,
                        )
                        nc.scalar.activation(
                            as_[0:Mh, h0:h0 + CW], ys[0:Mh, h0:h0 + CW], Act.Tanh
                        )
                        nc.vector.tensor_scalar(
                            zs[0:Mh, h0:h0 + CW],
                            ys[0:Mh, h0:h0 + CW],
                            0.0,
                            None,
                            mybir.AluOpType.is_gt,
                        )
                        if last:
                            # fan the tail DMAs across all queues
                            csl = slice(c0, c0 + CW)
                            hsl = slice(h0, h0 + CW)
                            engs = [
                                (nc.sync, nc.scalar, nc.gpsimd),
                                (nc.scalar, nc.gpsimd, nc.sync),
                                (nc.gpsimd, nc.sync, nc.scalar),
                                (nc.sync, nc.scalar, nc.gpsimd),
                            ][n]
                            engs[0].dma_start(
                                y_d[t0:t0 + My, csl], ys[lo:lo + My, hsl]
                            )
                            engs[1].dma_start(
                                a_d[t0:t0 + My, csl], as_[lo:lo + My, hsl]
                            )
                            engs[2].dma_start(
                                z_d[t0:t0 + My, csl], zs[lo:lo + My, hsl]
                            )
                        elif n % 2 == 1:
                            csl = slice(c0 - CW, c0 + CW)
                            nc.sync.dma_start(
                                y_d[t0:t0 + My, csl], ys[lo:lo + My, :]
                            )
                            nc.scalar.dma_start(
                                a_d[t0:t0 + My, csl], as_[lo:lo + My, :]
                            )
                            nc.gpsimd.dma_start(
                                z_d[t0:t0 + My, csl], zs[lo:lo + My, :]
                            )

    if split_waits:
        _split_matmul_waits(nc)
    return nc


def _get_nc(use_bias=True):
    key = ("nc", bool(use_bias))
    if key not in _CACHE:
        _CACHE[key] = _build_nc(bool(use_bias))
    return _CACHE[key]


def _prepare(x, W_dense, b_dense):
    x = np.asarray(x, np.float32)
    W = np.ascontiguousarray(np.asarray(W_dense, np.float32))
    b = np.asarray(b_dense, np.float32)
    use_bias = bool(np.any(b != 0.0))
    cst = _build_consts(b)
    in_maps = [
        {"x": np.ascontiguousarray(x[n]), "w": W, "cst": cst}
        for n in range(NCORES)
    ]
    return in_maps, use_bias


def kernel(x, W_dense, b_dense):
    from concourse.bass_utils import run_bass_kernel_spmd

    in_maps, use_bias = _prepare(x, W_dense, b_dense)
    nc = _get_nc(use_bias)
    res = run_bass_kernel_spmd(nc, in_maps, list(range(NCORES))).results

    y = np.stack([res[n]["y"] for n in range(NCORES)])
    a = np.stack([res[n]["a"] for n in range(NCORES)])
    z = np.stack([res[n]["z"] for n in range(NCORES)])
    return y, a, z


# revision 24
# speedup vs baseline: 1.0017x; 1.0017x over previous
"""Trainium2 Bass kernel for nn_ConvShiftLayer.

Computes, per batch element n:
    h = x[n] @ W_dense + b_dense                      (2048, 2048)
    y[t, o] = sum_{d=0..7} h[t-d, (o+d) % 2048]       (h[<0] = 0)
    a = tanh(y),  z = (y > 0) as f32
Returns (y, a, z) each of shape (8, 2048, 2048) f32.

Strategy: data-parallel over batch, 1 element per NeuronCore (8 cores).
Per core:
  - x is PE-transposed to xT tiles (D on partitions), 4 transposes per
    PSUM bank, copies split across scalar/vector engines.
  - h = xT.T @ W via fp32r matmuls (full-rate) into PSUM, copied to SBUF
    by the Scalar engine (activation Copy, f32r out); tiles hold 128
    overlapping time rows (stride 121) plus 1 wrap column.
  - The 8-tap shifted sum is a 3-stage shift tree:
        s1 = h  + Z h      (Z = time -1, chan +1)
        s2 = s1 + Z^2 s1
        y  = s2 + Z^4 s2 (+ bias)
    Each stage: 1 PE matmul for the shifted term (shift matrix
    stationary, channel shift via rhs free offset) + 1 DVE add for the
    identity term.  Cuts tap PE cost from 9 to 3-4 matmuls per chunk.
  - a = tanh on ScalarE, z = is_gt on DVE (2x SBUF mode).
  - Outputs staged in [128, 1024] pair-tiles -> 4 KB DMA lines, issued
    from sync (y), scalar (a) and gpsimd (z) queues; last tile fans its
    chunk DMAs across all four queues to shorten the tail drain.
"""

import sys

if "/opt/trn_rl_repo" not in sys.path:
    sys.path.insert(0, "/opt/trn_rl_repo")

import numpy as np

B, L, DIN, F = 8, 2048, 1024, 2048
WC = 8            # conv taps
PAD = WC - 1      # 7
TS = 128 - PAD    # 121 output rows per time tile
NT = (L + TS - 1) // TS   # 17 time tiles
NCH = 4           # channel chunks of 512
CW = 512          # chunk width
NCORES = 8
HW_ = F + 4       # h/s1/s2 tile width (max wrap need is s2: +4)

# consts tensor column layout (one [128, 2816] f32 input)
#   [0:128)      T1 down-shift by 1  (T1[m-1, m] = 1)
#   [128:256)    T2 down-shift by 2
#   [256:384)    T4 down-shift by 4
#   [384:512)    ones8 block  (rows 0..7 = 1)       -- bias, tiles i>0
#   [512:640)    L0 triangular (row d: 1 for cols >= d) -- bias, tile 0
#   [640:768)    identity (PE transpose)
#   [768:2816)   bsh: row d = bias shifted by d (rows 8..127 zero)
CONST_COLS = 768 + F

_CACHE = {}


def _build_consts(b):
    c = np.zeros((128, CONST_COLS), np.float32)
    for sh, base in ((1, 0), (2, 128), (4, 256)):
        for m in range(sh, 128):
            c[m - sh, base + m] = 1.0
    c[0:WC, 384:512] = 1.0
    for d in range(WC):
        c[d, 512 + d:640] = 1.0
    c[:, 640:768] = np.eye(128, dtype=np.float32)
    bext = np.concatenate([b, b[: WC - 1]])
    for d in range(WC):
        c[d, 768:768 + F] = bext[d:d + F]
    return c


def _split_matmul_waits(nc):
    """This walrus build accepts only one sync-wait command per instruction;
    hoist extra waits onto preceding same-engine no-ops (one wait each)."""
    import concourse.mybir as mybir

    for fn in nc.m.functions:
        for blk in fn.blocks:
            newl = []
            for inst in blk.instructions:
                si = getattr(inst, "sync_info", None)
                if (
                    si is not None
                    and len(si.on_wait) > 1
                    and not isinstance(inst, mybir.InstNoOp)
                    and getattr(inst, "engine", None) is not None
                ):
                    waits = list(si.on_wait)
                    for wi, w in enumerate(waits[:-1]):
                        pre = mybir.InstNoOp(
                            name=f"{inst.name}_wsplit{wi}",
                            sync_info=mybir.SyncInfo(on_wait=[w], on_update=[]),
                            bass_nofuse=True,
                            engine=inst.engine,
                        )
                        newl.append(pre)
                    si.on_wait = waits[-1:]
                newl.append(inst)
            blk.instructions = newl


def _build_nc(use_bias, mm_dtype_name="float32r", split_waits=True):
    import concourse.bass as bass
    import concourse.mybir as mybir
    from concourse import tile

    f32 = mybir.dt.float32
    mmdt = getattr(mybir.dt, mm_dtype_name)
    Act = mybir.ActivationFunctionType

    nc = bass.Bass("TRN2", target_bir_lowering=False, debug=False)

    x_d = nc.declare_dram_parameter("x", [L, DIN], f32, isOutput=False)
    w_d = nc.declare_dram_parameter("w", [DIN, F], f32, isOutput=False)
    cst_d = nc.declare_dram_parameter("cst", [128, CONST_COLS], f32, isOutput=False)
    y_d = nc.declare_dram_parameter("y", [L, F], f32, isOutput=True)
    a_d = nc.declare_dram_parameter("a", [L, F], f32, isOutput=True)
    z_d = nc.declare_dram_parameter("z", [L, F], f32, isOutput=True)

    KD = DIN // 128  # 8 K-tiles
    CCOLS = CONST_COLS if use_bias else 768

    with tile.TileContext(nc) as tc:
        with (
            tc.tile_pool(name="wpool", bufs=1) as wpool,
            tc.tile_pool(name="xtpool", bufs=1) as xtpool,
            tc.tile_pool(name="cpool", bufs=1) as cpool,
        ):
            cst = cpool.tile([128, CCOLS], mmdt, tag="cst", name="cst")
            wt = [wpool.tile([128, F], mmdt, tag=f"w{k}", name=f"w{k}") for k in range(KD)]
            xt = [
                xtpool.tile([128, L], mmdt, tag=f"xt{k}", name=f"xt{k}")
                for k in range(KD)
            ]

            ident = cst[:, 640:768].bitcast(f32)

            # ---- phase 0: transpose x (L,DIN) -> xT tiles [128 D, L t] ----
            with (
                tc.tile_pool(name="xstage", bufs=3) as xstage,
                tc.tile_pool(name="psum0", bufs=2, space="PSUM") as psum0,
            ):
                for i in range(L // 128):
                    xs = xstage.tile([128, DIN], f32, tag="xs")
                    if i == 0:
                        nc.sync.dma_start(xs[:, 0:512], x_d[0:128, 0:512])
                        nc.scalar.dma_start(
                            xs[:, 512:DIN], x_d[0:128, 512:DIN]
                        )
                    else:
                        nc.sync.dma_start(xs[:], x_d[i * 128:(i + 1) * 128, :])
                    if i == 0:
                        # W/cst loads issued after x0 so transposes start
                        # immediately; they ride parallel queues.
                        nc.gpsimd.dma_start(
                            cst[:], cst_d[:, 0:CCOLS].bitcast(mmdt)
                        )
                        for k in range(KD):
                            nc.scalar.dma_start(
                                wt[k][:],
                                w_d[k * 128:(k + 1) * 128, :].bitcast(mmdt),
                            )
                    for g in range(2):  # groups of 4 transposes per bank
                        tp = psum0.tile([128, 512], f32, tag="tp")
                        for j in range(4):
                            k = g * 4 + j
                            nc.tensor.transpose(
                                tp[:, j * 128:(j + 1) * 128],
                                xs[:, k * 128:(k + 1) * 128],
                                ident,
                            )
                        for j in range(4):
                            k = g * 4 + j
                            dst = xt[k][:, i * 128:(i + 1) * 128]
                            src = tp[:, j * 128:(j + 1) * 128]
                            if k % 2 == 0:
                                nc.scalar.activation(dst, src, Act.Copy)
                            else:
                                nc.vector.tensor_copy(dst, src)

            # ---- phase 1: software-pipelined over time tiles ----
            # Emission order: dense matmuls of tile ii, then tap stages of
            # tile ii-1.  Keeps ~8us of independent dense work queued ahead
            # of every cross-engine-dependent stage matmul so the in-order
            # PE queue never stalls on DVE/scalar sems (head-of-line block).
            with (
                tc.tile_pool(name="hring", bufs=4) as hring,
                tc.tile_pool(name="opool", bufs=3) as opool,
                tc.tile_pool(name="psum_h", bufs=3, space="PSUM") as psum_h,
                tc.tile_pool(name="psum_m", bufs=5, space="PSUM") as psum_m,
            ):
                def tile_geom(i):
                    t0 = TS * i
                    My = min(TS, L - t0)
                    if i == 0:
                        return t0, My, 0, 128, 0
                    hlo = t0 - PAD
                    Mh = min(L, t0 + TS) - hlo
                    return t0, My, hlo, Mh, PAD

                for i in range(NT):
                    t0, My, hlo, Mh, lo = tile_geom(i)
                    hs = hring.tile([128, HW_], f32, tag="h", name=f"hs{i}")
                    s1 = hring.tile([128, HW_], f32, tag="h", name=f"s1_{i}")
                    s2 = hring.tile([128, HW_], f32, tag="h", name=f"s2_{i}")
                    hsr = hs.bitcast(mmdt)
                    s1r = s1.bitcast(mmdt)
                    s2r = s2.bitcast(mmdt)

                    # h = xT.T @ W ; PSUM -> SBUF on scalar engine
                    for n in range(NCH):
                        hp = psum_h.tile([128, CW], f32, tag="hp")
                        for k in range(KD):
                            nc.tensor.matmul(
                                hp[0:Mh, :],
                                xt[k][:, hlo:hlo + Mh],
                                wt[k][:, n * CW:(n + 1) * CW],
                                start=(k == 0),
                                stop=(k == KD - 1),
                            )
                        nc.scalar.activation(
                            hsr[0:Mh, n * CW:(n + 1) * CW], hp[0:Mh, :], Act.Copy
                        )
                        if n == 0:  # wrap col: hs[:, F] = hs[:, 0]
                            nc.scalar.activation(
                                hsr[0:Mh, F:F + 1], hsr[0:Mh, 0:1], Act.Copy
                            )

                    # stage 1: s1 = hs + Z hs
                    for n in range(NCH):
                        c0 = n * CW
                        m1 = psum_m.tile([128, CW], f32, tag="m")
                        nc.tensor.matmul(
                            m1[0:Mh, :],
                            cst[0:Mh, 0:Mh],
                            hsr[0:Mh, c0 + 1:c0 + 1 + CW],
                            start=True,
                            stop=True,
                        )
                        nc.vector.tensor_tensor(
                            s1r[0:Mh, c0:c0 + CW],
                            hs[0:Mh, c0:c0 + CW],
                            m1[0:Mh, :],
                            mybir.AluOpType.add,
                        )
                        if n == 0:  # wrap cols: s1[:, F:F+2] = s1[:, 0:2]
                            nc.scalar.activation(
                                s1r[0:Mh, F:F + 2], s1r[0:Mh, 0:2], Act.Copy
                            )

                    # stage 2: s2 = s1 + Z^2 s1
                    for n in range(NCH):
                        c0 = n * CW
                        m2 = psum_m.tile([128, CW], f32, tag="m")
                        nc.tensor.matmul(
                            m2[0:Mh, :],
                            cst[0:Mh, 128:128 + Mh],
                            s1r[0:Mh, c0 + 2:c0 + 2 + CW],
                            start=True,
                            stop=True,
                        )
                        nc.vector.tensor_tensor(
                            s2r[0:Mh, c0:c0 + CW],
                            s1[0:Mh, c0:c0 + CW],
                            m2[0:Mh, :],
                            mybir.AluOpType.add,
                        )
                        if n == 0:  # wrap cols: s2[:, F:F+4] = s2[:, 0:4]
                            nc.scalar.activation(
                                s2r[0:Mh, F:F + 4], s2r[0:Mh, 0:4], Act.Copy
                            )

                    # stage 3: y = s2 + Z^4 s2 (+ bias); tanh; is_gt; DMA out
                    bias_col = 512 if i == 0 else 384
                    last = i >= NT - 3
                    ys = as_ = zs = None
                    for n in range(NCH):
                        c0 = n * CW
                        if n % 2 == 0:
                            ys = opool.tile([128, 2 * CW], f32, tag="ys", name="ys")
                            as_ = opool.tile([128, 2 * CW], f32, tag="as", name="as_")
                            zs = opool.tile([128, 2 * CW], f32, tag="zs", name="zs")
                        h0 = (n % 2) * CW
                        m3 = psum_m.tile([128, CW], f32, tag="m")
                        nc.tensor.matmul(
                            m3[0:Mh, :],
                            cst[0:Mh, 256:256 + Mh],
                            s2r[0:Mh, c0 + 4:c0 + 4 + CW],
                            start=True,
                            stop=not use_bias,
                        )
                        if use_bias:
                            nc.tensor.matmul(
                                m3[0:Mh, :],
                                cst[0:Mh, bias_col:bias_col + Mh],
                                cst[0:Mh, 768 + c0:768 + c0 + CW],
                                start=False,
                                stop=True,
                            )
                        nc.vector.tensor_tensor(
                            ys[0:Mh, h0:h0 + CW],
                            s2[0:Mh, c0:c0 + CW],
                            m3[0:Mh, :],
                            mybir.AluOpType.add,
                        )
                        nc.scalar.activation(
                            as_[0:Mh, h0:h0 + CW], ys[0:Mh, h0:h0 + CW], Act.Tanh
                        )
                        nc.vector.tensor_scalar(
                            zs[0:Mh, h0:h0 + CW],
                            ys[0:Mh, h0:h0 + CW],
                            0.0,
                            None,
                            mybir.AluOpType.is_gt,
                        )
                        if last:
                            # fan the tail DMAs across all queues
                            csl = slice(c0, c0 + CW)
                            hsl = slice(h0, h0 + CW)
                            engs = [
                                (nc.sync, nc.scalar, nc.gpsimd),
                                (nc.scalar, nc.gpsimd, nc.sync),
                                (nc.gpsimd, nc.sync, nc.scalar),
                                (nc.sync, nc.scalar, nc.gpsimd),
                            ][n]
                            engs[0].dma_start(
                                y_d[t0:t0 + My, csl], ys[lo:lo + My, hsl]
                            )
                            engs[1].dma_start(
                                a_d[t0:t0 + My, csl], as_[lo:lo + My, hsl]
                            )
                            engs[2].dma_start(
                                z_d[t0:t0 + My, csl], zs[lo:lo + My, hsl]
                            )
                        elif n % 2 == 1:
                            csl = slice(c0 - CW, c0 + CW)
                            nc.sync.dma_start(
                                y_d[t0:t0 + My, csl], ys[lo:lo + My, :]
                            )
                            nc.scalar.dma_start(
                                a_d[t0:t0 + My, csl], as_[lo:lo + My, :]
                            )
                            nc.gpsimd.dma_start(
                                z_d[t0:t0 + My, csl], zs[lo:lo + My, :]
                            )

    if split_waits:
        _split_matmul_waits(nc)
    return nc


def _get_nc(use_bias=True):
    key = ("nc", bool(use_bias))
    if key not in _CACHE:
        _CACHE[key] = _build_nc(bool(use_bias))
    return _CACHE[key]


def _prepare(x, W_dense, b_dense):
    x = np.asarray(x, np.float32)
    W = np.ascontiguousarray(np.asarray(W_dense, np.float32))
    b = np.asarray(b_dense, np.float32)
    use_bias = bool(np.any(b != 0.0))
    cst = _build_consts(b)
    in_maps = [
        {"x": np.ascontiguousarray(x[n]), "w": W, "cst": cst}
        for n in range(NCORES)
    ]
    return in_maps, use_bias


def kernel(x, W_dense, b_dense):
    from concourse.bass_utils import run_bass_kernel_spmd

    in_maps, use_bias = _prepare(x, W_dense, b_dense)
    nc = _get_nc(use_bias)
    res = run_bass_kernel_spmd(nc, in_maps, list(range(NCORES))).results

    y = np.stack([res[n]["y"] for n in range(NCORES)])
    a = np.stack([res[n]["a"] for n in range(NCORES)])
    z = np.stack([res[n]["z"] for n in range(NCORES)])
    return y, a, z
